# revision 42
# baseline (speedup 1.0000x reference)
"""Trainium2 Bass kernel for a binary (1w1a) depthwise-separable conv block.

Reference computation (NCHW, B=32, C=CO=512, H=W=56):
    xb  = sign(x)
    y1  = depthwise_conv3x3(xb, sign(w_dw), pad=1)          # per-channel
    z   = sign(y1 * s1 + t1)                                # BN1 + binarize
    y2  = pointwise_conv1x1(z, sign(w_pw))                  # dense 512->512
    out = y2 * s2 + t2                                      # BN2

Sharding: data-parallel over batch, 4 images per core on 8 cores.

All intermediate values are {-1, 0, +1}; products and the <=512-term fp32 PSUM
accumulations are exact in fp8/bf16, so the result matches fp32 reference
numerics except for the final BN2 affine (done in fp32) and ~1-ulp BN-constant
rounding.

Device mapping (per core, per image/channel-group):
  - sign(x)          -> ScalarE Sign LUT, fp32 -> fp8 into slot 0 of a
                        zero-padded [128, 2, 60, 60] buffer (60-row pitch, two
                        "DoubleRow" slots).  Slot 1 = slot 0 shifted one row
                        (+60 elements), produced by a VectorE bf16-bitcast copy.
  - depthwise conv   -> TensorE fp8 DoubleRow: 6 accumulating matmuls per
                        8-row output chunk. Each pass contracts 2 taps at once:
                        pairs (dh=0,dw)+(dh=1,dw) via the slot-1 row shift, and
                        (dh=2,dw) paired with a zero diagonal.  Stationary =
                        [128, 2, 128] diagonal pair.
  - BN1 + sign       -> ScalarE: z = Sign(scale*psum + bias) -> fp8, written
                        into [128, 2, 56, 56] z-pair tiles (slot = channel grp).
  - pointwise conv   -> TensorE fp8 DoubleRow: 2 accumulating passes contract
                        all 512 input channels (2 channel groups per pass).
  - BN2 + evict      -> VectorE tensor_scalar: psum*s2 + t2 -> fp32 SBUF.
"""

import sys

sys.path.insert(0, "/opt/trn_rl_repo")

from contextlib import ExitStack

import ml_dtypes
import numpy as np

import concourse.bass as bass
import concourse.tile as tile
from concourse import mybir
from concourse.bass_utils import run_bass_kernel_spmd

N_CORES = 8
B, C, H, W = 32, 512, 56, 56
CO = 512
EPS = 1e-5
BS = B // N_CORES          # images per core
CG = C // 128              # channel groups
ROWS = 8                   # output rows per PSUM chunk (8*56=448 fp32 <= 1 bank)
NCHUNK = H // ROWS         # 7
PH, PW_ = 60, 60           # padded buffer pitch: rows 0/57..59 and cols 0/57..59 zero

F32 = mybir.dt.float32
FP8 = mybir.dt.float8e4
DR = mybir.MatmulPerfMode.DoubleRow
NP_FP8 = ml_dtypes.float8_e4m3


def _legalize_sem_waits(nc, max_waits=1):
    """walrus (CoreV3 codegen) rejects instructions carrying more than one
    sync-wait command.  Tile's kernel-tail drain waits on every outstanding
    semaphore at once; split excess waits onto preceding no-ops on the same
    engine (engines execute their stream in order, so blocking semantics are
    identical)."""
    n_split = 0
    for f in nc.m.functions:
        for bb in f.blocks:
            insts = bb.instructions
            newlist = []
            for inst in insts:
                si = inst.sync_info
                waits = list(si.on_wait) if si is not None else []
                if len(waits) > max_waits:
                    excess, keep = waits[:-max_waits], waits[-max_waits:]
                    for k, w in enumerate(excess):
                        sp = mybir.InstNoOp(name=f"{inst.name}-lgw{k}")
                        sp.engine = inst.engine
                        sp.sync_info = mybir.SyncInfo(on_wait=[w], on_update=[])
                        newlist.append(sp)
                        n_split += 1
                    inst.sync_info = mybir.SyncInfo(
                        on_wait=keep, on_update=list(si.on_update)
                    )
                newlist.append(inst)
            insts[:] = newlist
    return n_split


def build_bass():
    nc = bass.Bass("TRN2", target_bir_lowering=False, debug=False)

    x_d = nc.dram_tensor("x", [BS, C, H, W], F32, kind="ExternalInput")
    # dw pairs: idx = cg*5 + p; p in 0..2 -> taps (0,p)&(1,p) [buffer A,
    # slot1=+1row]; p=3 -> taps (2,0)&(2,2) [buffer B, slot1=+2cols];
    # p=4 -> tap (2,1) & zero [buffer A]
    wdw_d = nc.dram_tensor("wdw", [128, CG * 5, 2, 128], FP8, kind="ExternalInput")
    # pw pairs: idx = zpair*CG + cob; slot j of zpair holds channels
    # (zpair*2+j)*128 ..
    wpw_d = nc.dram_tensor("wpw", [128, 2 * CG, 2, 128], FP8, kind="ExternalInput")
    bn1_d = nc.dram_tensor("bn1", [128, 2 * CG], F32, kind="ExternalInput")
    bn2_d = nc.dram_tensor("bn2", [128, 2 * CG], F32, kind="ExternalInput")
    y_d = nc.dram_tensor("y", [BS, CO, H, W], F32, kind="ExternalOutput")

    SIGN = mybir.ActivationFunctionType.Sign
    MULT = mybir.AluOpType.mult
    ADD = mybir.AluOpType.add

    with tile.TileContext(nc) as tc:
        with ExitStack() as ctx:
            const = ctx.enter_context(tc.tile_pool(name="const", bufs=1))
            xin_pool = ctx.enter_context(tc.tile_pool(name="xin", bufs=6))

            # Prefetch the whole first image before the (bulkier) weight DMAs
            # so the Scalar/Vector/PE pipeline can start ASAP.  Input DMAs for
            # image b+1 are always issued before image b's output DMAs enter
            # the Sync queue: output issue blocks on BN2 evictions, and
            # sharing the FIFO position would stall the next image's loads at
            # every image boundary.
            xin_tiles = {}
            # first tile arrives in two halves so Sign/PE can start ~3us in
            t = xin_pool.tile([128, H, W], F32, tag="xin")
            nc.sync.dma_start(t[:, 0:28, :], x_d.ap()[0, 0:128][:, 0:28, :])
            nc.sync.dma_start(t[:, 28:H, :], x_d.ap()[0, 0:128][:, 28:H, :])
            xin_tiles[(0, 0)] = t

            wdw_t = const.tile([128, CG * 5, 2, 128], FP8, tag="wdw")
            for wcg in range(CG):
                nc.sync.dma_start(
                    wdw_t[:, wcg * 5 : (wcg + 1) * 5],
                    wdw_d.ap()[:, wcg * 5 : (wcg + 1) * 5],
                )
            for pcg in range(1, CG):
                t = xin_pool.tile([128, H, W], F32, tag="xin")
                nc.sync.dma_start(t[:], x_d.ap()[0, pcg * 128 : (pcg + 1) * 128])
                xin_tiles[(0, pcg)] = t
            wpw_t = const.tile([128, 2 * CG, 2, 128], FP8, tag="wpw")
            nc.sync.dma_start(wpw_t[:], wpw_d.ap()[:])
            bn1_t = const.tile([128, 2 * CG], F32, tag="bn1")
            nc.sync.dma_start(bn1_t[:], bn1_d.ap()[:])
            bn2_t = const.tile([128, 2 * CG], F32, tag="bn2")
            nc.sync.dma_start(bn2_t[:], bn2_d.ap()[:])

            # persistent padded sign(x) buffers: [slot, 60, 60], borders zero.
            # A: slot1 = slot0 shifted +60 (one row).  B: slot1 = slot0
            # shifted +2 (two cols); B is fully rewritten by copies each use,
            # so only A needs the one-time zero fill.
            xpads = []
            for k in range(3):
                xpa = const.tile([128, 2, PH, PW_], FP8, tag=f"xpada{k}")
                # one-time zero on GpSimd (idle) via a uint32 view; B buffers
                # need no fill — the per-iteration copies rewrite every byte
                # that is ever read from them
                xp32 = xpa[:].rearrange("p a b c -> p (a b c)").bitcast(
                    mybir.dt.uint32
                )
                nc.gpsimd.memset(xp32, 0)
                xpb = const.tile([128, 2, PH, PW_], FP8, tag=f"xpadb{k}")
                xpads.append((xpa, xpb))

            z_pool = ctx.enter_context(tc.tile_pool(name="z", bufs=4))
            out_pool = ctx.enter_context(tc.tile_pool(name="outb", bufs=2))
            psdw_pool = ctx.enter_context(
                tc.tile_pool(name="psdw", bufs=2, space="PSUM")
            )
            pspw_pool = ctx.enter_context(
                tc.tile_pool(name="pspw", bufs=4, space="PSUM")
            )

            it = 0
            for b in range(BS):
                # prefetch next image's inputs ahead of this image's outputs
                if b + 1 < BS:
                    for pcg in range(CG):
                        t = xin_pool.tile([128, H, W], F32, tag="xin")
                        nc.sync.dma_start(
                            t[:], x_d.ap()[b + 1, pcg * 128 : (pcg + 1) * 128]
                        )
                        xin_tiles[(b + 1, pcg)] = t
                zp = []
                for _zi in range(2):
                    ztile = z_pool.tile([128, 2, H, W], FP8, tag="z")
                    zp.append(ztile)
                for cg in range(CG):
                    xin = xin_tiles.pop((b, cg))

                    xpa, xpb = xpads[it % 3]
                    it += 1
                    # A slot 0 interior = sign(x); split for the very first
                    # tile so PE work begins before the full tile lands
                    if b == 0 and cg == 0:
                        nc.scalar.activation(
                            xpa[:, 0, 1:29, 1 : W + 1], xin[:, 0:28, :], SIGN
                        )
                        nc.scalar.activation(
                            xpa[:, 0, 29 : H + 1, 1 : W + 1], xin[:, 28:H, :], SIGN
                        )
                    else:
                        nc.scalar.activation(
                            xpa[:, 0, 1 : H + 1, 1 : W + 1], xin[:], SIGN
                        )
                    # bf16-bitcast flat views so copies run in fast DVE modes
                    fa = xpa[:].rearrange("p a b c -> p (a b c)").bitcast(
                        mybir.dt.bfloat16
                    )  # [128, 3600]
                    fb = xpb[:].rearrange("p a b c -> p (a b c)").bitcast(
                        mybir.dt.bfloat16
                    )
                    # A slot1 = A slot0 shifted +60 fp8 (one row)
                    nc.vector.tensor_copy(fa[:, 1800:3540], fa[:, 30:1770])
                    # B slot0 = A slot0;  B slot1 = A slot0 shifted +2 fp8
                    nc.vector.tensor_copy(fb[:, 0:1740], fa[:, 0:1740])
                    nc.vector.tensor_copy(fb[:, 1800:3540], fa[:, 1:1741])

                    zslot, j = zp[cg // 2], cg % 2
                    # chunk pairs share one 2-bank PSUM tile so the BN1+Sign
                    # eviction reads 2 banks in a single ScalarE op.  Pass
                    # loop is OUTER so each stationary serves both members
                    # back-to-back (halves LDWEIGHTS traffic).
                    for pg in range(4):
                        members = [2 * pg, 2 * pg + 1] if pg < 3 else [6]
                        ps2 = psdw_pool.tile([128, 2, 512], F32, tag="psdw")
                        # (weight idx, buffer, row off, col off) per pass
                        passes = [
                            (cg * 5 + 0, xpa, 0, 0),
                            (cg * 5 + 1, xpa, 0, 1),
                            (cg * 5 + 2, xpa, 0, 2),
                            (cg * 5 + 3, xpb, 2, 0),
                            (cg * 5 + 4, xpa, 2, 1),
                        ]
                        for p, (wi, buf, ro, co) in enumerate(passes):
                            for s, n in enumerate(members):
                                r0 = n * ROWS + ro
                                nc.tensor.matmul(
                                    ps2[:, s, 0 : ROWS * W],
                                    wdw_t[:, wi],
                                    buf[:, :, r0 : r0 + ROWS, co : co + W],
                                    start=(p == 0),
                                    stop=(p == 4),
                                    perf_mode=DR,
                                )
                        r0 = members[0] * ROWS
                        nrows = ROWS * len(members)
                        zout = zslot[:, j, r0 : r0 + nrows, :].rearrange(
                            "p (a r) w -> p a (r w)", a=len(members)
                        )
                        nc.scalar.activation(
                            zout,
                            ps2[:, 0 : len(members), 0 : ROWS * W],
                            SIGN,
                            bias=bn1_t[:, cg * 2 + 1 : cg * 2 + 2],
                            scale=bn1_t[:, cg * 2 : cg * 2 + 1],
                        )

                for cob in range(CG):
                    outb = out_pool.tile([128, H, W], F32, tag="outb")
                    for n in range(NCHUNK):
                        pp = pspw_pool.tile([128, 512], F32, tag="pspw")
                        r0 = n * ROWS
                        for zpair in range(2):
                            nc.tensor.matmul(
                                pp[:, 0 : ROWS * W],
                                wpw_t[:, zpair * CG + cob],
                                zp[zpair][:, :, r0 : r0 + ROWS, :],
                                start=(zpair == 0),
                                stop=(zpair == 1),
                                perf_mode=DR,
                            )
                        oout = outb[:, r0 : r0 + ROWS, :].rearrange(
                            "p r w -> p (r w)"
                        )
                        nc.vector.tensor_scalar(
                            oout,
                            pp[:, 0 : ROWS * W],
                            bn2_t[:, cob * 2 : cob * 2 + 1],
                            bn2_t[:, cob * 2 + 1 : cob * 2 + 2],
                            MULT,
                            ADD,
                        )
                        # stream the output out in halves (rows 0:32 after the
                        # fourth chunk, rest at the end) so the final drain
                        # overlaps compute
                        if n == 3:
                            nc_half = y_d.ap()[b, cob * 128 : (cob + 1) * 128]
                            nc.sync.dma_start(nc_half[:, 0:32, :], outb[:, 0:32, :])
                    nc.sync.dma_start(
                        y_d.ap()[b, cob * 128 : (cob + 1) * 128][:, 32:H, :],
                        outb[:, 32:H, :],
                    )

    _legalize_sem_waits(nc)
    return nc


_NC_CACHE = None


def _get_nc():
    global _NC_CACHE
    if _NC_CACHE is None:
        _NC_CACHE = build_bass()
    return _NC_CACHE


def make_host_inputs(w_dw, w_pw, g1, b1, m1, v1, g2, b2, m2, v2):
    """Host-side preprocessing shared by all cores (weights/BN constants)."""
    wsign = np.sign(w_dw[:, 0, :, :]).reshape(C, 3, 3).astype(np.float32)

    wdw = np.zeros((128, CG * 5, 2, 128), dtype=NP_FP8)
    idx = np.arange(128)
    for cg in range(CG):
        cs = slice(cg * 128, (cg + 1) * 128)
        for dw in range(3):
            wdw[idx, cg * 5 + dw, 0, idx] = wsign[cs, 0, dw].astype(NP_FP8)
            wdw[idx, cg * 5 + dw, 1, idx] = wsign[cs, 1, dw].astype(NP_FP8)
        # pair 3 (buffer B): slot0 = tap (2,0), slot1 = tap (2,2)
        wdw[idx, cg * 5 + 3, 0, idx] = wsign[cs, 2, 0].astype(NP_FP8)
        wdw[idx, cg * 5 + 3, 1, idx] = wsign[cs, 2, 2].astype(NP_FP8)
        # pair 4 (buffer A): slot0 = tap (2,1), slot1 stays zero
        wdw[idx, cg * 5 + 4, 0, idx] = wsign[cs, 2, 1].astype(NP_FP8)

    wptT = np.sign(w_pw[:, :, 0, 0]).T.astype(np.float32)  # [c, co]
    wpw = np.zeros((128, 2 * CG, 2, 128), dtype=NP_FP8)
    for zpair in range(2):
        for cob in range(CG):
            for j in range(2):
                c0 = (zpair * 2 + j) * 128
                wpw[:, zpair * CG + cob, j, :] = wptT[
                    c0 : c0 + 128, cob * 128 : (cob + 1) * 128
                ].astype(NP_FP8)

    def bn_consts(g, bta, m, v):
        s = (g.astype(np.float64) / np.sqrt(v.astype(np.float64) + EPS)).astype(
            np.float32
        )
        t = bta.astype(np.float32) - m.astype(np.float32) * s
        out = np.zeros((128, 2 * CG), dtype=np.float32)
        for cg in range(CG):
            out[:, cg * 2] = s[cg * 128 : (cg + 1) * 128]
            out[:, cg * 2 + 1] = t[cg * 128 : (cg + 1) * 128]
        return out

    return {
        "wdw": wdw,
        "wpw": wpw,
        "bn1": bn_consts(g1, b1, m1, v1),
        "bn2": bn_consts(g2, b2, m2, v2),
    }


def kernel(x, w_dw, w_pw, g1, b1, m1, v1, g2, b2, m2, v2, _trace=False, _tmpdir=None):
    x = np.asarray(x, dtype=np.float32)
    shared = make_host_inputs(
        np.asarray(w_dw), np.asarray(w_pw),
        np.asarray(g1), np.asarray(b1), np.asarray(m1), np.asarray(v1),
        np.asarray(g2), np.asarray(b2), np.asarray(m2), np.asarray(v2),
    )
    in_maps = []
    for i in range(N_CORES):
        m = {"x": np.ascontiguousarray(x[i * BS : (i + 1) * BS])}
        m.update(shared)
        in_maps.append(m)

    nc = _get_nc()
    res = run_bass_kernel_spmd(
        nc, in_maps, core_ids=list(range(N_CORES)), trace=_trace, tmpdir=_tmpdir
    )
    y = np.concatenate([res.results[i]["y"] for i in range(N_CORES)], axis=0)
    if _trace:
        return y, res
    return y


# revision 43
# speedup vs baseline: 1.0453x; 1.0453x over previous
"""Trainium2 Bass kernel for a binary (1w1a) depthwise-separable conv block.

Reference computation (NCHW, B=32, C=CO=512, H=W=56):
    xb  = sign(x)
    y1  = depthwise_conv3x3(xb, sign(w_dw), pad=1)          # per-channel
    z   = sign(y1 * s1 + t1)                                # BN1 + binarize
    y2  = pointwise_conv1x1(z, sign(w_pw))                  # dense 512->512
    out = y2 * s2 + t2                                      # BN2

Sharding: data-parallel over batch, 4 images per core on 8 cores.

All intermediate values are {-1, 0, +1}; products and the <=512-term fp32 PSUM
accumulations are exact in fp8/bf16, so the result matches fp32 reference
numerics except for the final BN2 affine (done in fp32) and ~1-ulp BN-constant
rounding.

Device mapping (per core, per image/channel-group):
  - sign(x)          -> ScalarE Sign LUT, fp32 -> fp8 into slot 0 of a
                        zero-padded [128, 2, 60, 60] buffer (60-row pitch, two
                        "DoubleRow" slots).  Slot 1 = slot 0 shifted one row
                        (+60 elements), produced by a VectorE bf16-bitcast copy.
  - depthwise conv   -> TensorE fp8 DoubleRow: 6 accumulating matmuls per
                        8-row output chunk. Each pass contracts 2 taps at once:
                        pairs (dh=0,dw)+(dh=1,dw) via the slot-1 row shift, and
                        (dh=2,dw) paired with a zero diagonal.  Stationary =
                        [128, 2, 128] diagonal pair.
  - BN1 + sign       -> ScalarE: z = Sign(scale*psum + bias) -> fp8, written
                        into [128, 2, 56, 56] z-pair tiles (slot = channel grp).
  - pointwise conv   -> TensorE fp8 DoubleRow: 2 accumulating passes contract
                        all 512 input channels (2 channel groups per pass).
  - BN2 + evict      -> VectorE tensor_scalar: psum*s2 + t2 -> fp32 SBUF.
"""

import sys

sys.path.insert(0, "/opt/trn_rl_repo")

from contextlib import ExitStack

import ml_dtypes
import numpy as np

import concourse.bass as bass
import concourse.tile as tile
from concourse import mybir
from concourse.bass_utils import run_bass_kernel_spmd

N_CORES = 8
B, C, H, W = 32, 512, 56, 56
CO = 512
EPS = 1e-5
BS = B // N_CORES          # images per core
CG = C // 128              # channel groups
ROWS = 8                   # output rows per PSUM chunk (8*56=448 fp32 <= 1 bank)
NCHUNK = H // ROWS         # 7
PH, PW_ = 60, 60           # padded buffer pitch: rows 0/57..59 and cols 0/57..59 zero

F32 = mybir.dt.float32
FP8 = mybir.dt.float8e4
DR = mybir.MatmulPerfMode.DoubleRow
NP_FP8 = ml_dtypes.float8_e4m3


def _legalize_sem_waits(nc, max_waits=1):
    """walrus (CoreV3 codegen) rejects instructions carrying more than one
    sync-wait command.  Tile's kernel-tail drain waits on every outstanding
    semaphore at once; split excess waits onto preceding no-ops on the same
    engine (engines execute their stream in order, so blocking semantics are
    identical)."""
    n_split = 0
    for f in nc.m.functions:
        for bb in f.blocks:
            insts = bb.instructions
            newlist = []
            for inst in insts:
                si = inst.sync_info
                waits = list(si.on_wait) if si is not None else []
                if len(waits) > max_waits:
                    excess, keep = waits[:-max_waits], waits[-max_waits:]
                    for k, w in enumerate(excess):
                        sp = mybir.InstNoOp(name=f"{inst.name}-lgw{k}")
                        sp.engine = inst.engine
                        sp.sync_info = mybir.SyncInfo(on_wait=[w], on_update=[])
                        newlist.append(sp)
                        n_split += 1
                    inst.sync_info = mybir.SyncInfo(
                        on_wait=keep, on_update=list(si.on_update)
                    )
                newlist.append(inst)
            insts[:] = newlist
    return n_split


def build_bass():
    nc = bass.Bass("TRN2", target_bir_lowering=False, debug=False)

    x_d = nc.dram_tensor("x", [BS, C, H, W], F32, kind="ExternalInput")
    # dw pairs: idx = cg*5 + p; p in 0..2 -> taps (0,p)&(1,p) [buffer A,
    # slot1=+1row]; p=3 -> taps (2,0)&(2,2) [buffer B, slot1=+2cols];
    # p=4 -> tap (2,1) & zero [buffer A]
    wdw_d = nc.dram_tensor("wdw", [128, CG * 5, 2, 128], FP8, kind="ExternalInput")
    # pw pairs: idx = zpair*CG + cob; slot j of zpair holds channels
    # (zpair*2+j)*128 ..
    wpw_d = nc.dram_tensor("wpw", [128, 2 * CG, 2, 128], FP8, kind="ExternalInput")
    bn1_d = nc.dram_tensor("bn1", [128, 2 * CG], F32, kind="ExternalInput")
    bn2_d = nc.dram_tensor("bn2", [128, 2 * CG], F32, kind="ExternalInput")
    y_d = nc.dram_tensor("y", [BS, CO, H, W], F32, kind="ExternalOutput")

    SIGN = mybir.ActivationFunctionType.Sign
    MULT = mybir.AluOpType.mult
    ADD = mybir.AluOpType.add

    with tile.TileContext(nc) as tc:
        with ExitStack() as ctx:
            const = ctx.enter_context(tc.tile_pool(name="const", bufs=1))
            xin_pool = ctx.enter_context(tc.tile_pool(name="xin", bufs=6))

            # Prefetch the whole first image before the (bulkier) weight DMAs
            # so the Scalar/Vector/PE pipeline can start ASAP.  Input DMAs for
            # image b+1 are always issued before image b's output DMAs enter
            # the Sync queue: output issue blocks on BN2 evictions, and
            # sharing the FIFO position would stall the next image's loads at
            # every image boundary.
            xin_tiles = {}
            # first tile arrives in two halves so Sign/PE can start ~3us in
            t = xin_pool.tile([128, H, W], F32, tag="xin")
            nc.sync.dma_start(t[:, 0:28, :], x_d.ap()[0, 0:128][:, 0:28, :])
            nc.sync.dma_start(t[:, 28:H, :], x_d.ap()[0, 0:128][:, 28:H, :])
            xin_tiles[(0, 0)] = t

            wdw_t = const.tile([128, CG * 5, 2, 128], FP8, tag="wdw")
            for wcg in range(CG):
                nc.sync.dma_start(
                    wdw_t[:, wcg * 5 : (wcg + 1) * 5],
                    wdw_d.ap()[:, wcg * 5 : (wcg + 1) * 5],
                )
            for pcg in range(1, CG):
                t = xin_pool.tile([128, H, W], F32, tag="xin")
                nc.sync.dma_start(t[:], x_d.ap()[0, pcg * 128 : (pcg + 1) * 128])
                xin_tiles[(0, pcg)] = t
            wpw_t = const.tile([128, 2 * CG, 2, 128], FP8, tag="wpw")
            nc.sync.dma_start(wpw_t[:], wpw_d.ap()[:])
            bn1_t = const.tile([128, 2 * CG], F32, tag="bn1")
            nc.sync.dma_start(bn1_t[:], bn1_d.ap()[:])
            bn2_t = const.tile([128, 2 * CG], F32, tag="bn2")
            nc.sync.dma_start(bn2_t[:], bn2_d.ap()[:])

            # persistent padded sign(x) buffers: [slot, 60, 60], borders zero.
            # A: slot1 = slot0 shifted +60 (one row).  B: slot1 = slot0
            # shifted +2 (two cols); B is fully rewritten by copies each use,
            # so only A needs the one-time zero fill.
            xpads = []
            for k in range(3):
                xpa = const.tile([128, 2, PH, PW_], FP8, tag=f"xpada{k}")
                # one-time zero on GpSimd (idle) via a uint32 view; B buffers
                # need no fill — the per-iteration copies rewrite every byte
                # that is ever read from them
                xp32 = xpa[:].rearrange("p a b c -> p (a b c)").bitcast(
                    mybir.dt.uint32
                )
                nc.vector.memset(xp32, 0)
                xpb = const.tile([128, 2, PH, PW_], FP8, tag=f"xpadb{k}")
                xpads.append((xpa, xpb))

            z_pool = ctx.enter_context(tc.tile_pool(name="z", bufs=4))
            out_pool = ctx.enter_context(tc.tile_pool(name="outb", bufs=2))
            psdw_pool = ctx.enter_context(
                tc.tile_pool(name="psdw", bufs=2, space="PSUM")
            )
            pspw_pool = ctx.enter_context(
                tc.tile_pool(name="pspw", bufs=4, space="PSUM")
            )

            it = 0
            for b in range(BS):
                # prefetch next image's inputs ahead of this image's outputs
                if b + 1 < BS:
                    for pcg in range(CG):
                        t = xin_pool.tile([128, H, W], F32, tag="xin")
                        nc.sync.dma_start(
                            t[:], x_d.ap()[b + 1, pcg * 128 : (pcg + 1) * 128]
                        )
                        xin_tiles[(b + 1, pcg)] = t
                zp = []
                for _zi in range(2):
                    ztile = z_pool.tile([128, 2, H, W], FP8, tag="z")
                    zp.append(ztile)
                for cg in range(CG):
                    xin = xin_tiles.pop((b, cg))

                    xpa, xpb = xpads[it % 3]
                    it += 1
                    # A slot 0 interior = sign(x); split for the very first
                    # tile so PE work begins before the full tile lands
                    if b == 0 and cg == 0:
                        nc.scalar.activation(
                            xpa[:, 0, 1:29, 1 : W + 1], xin[:, 0:28, :], SIGN
                        )
                        nc.scalar.activation(
                            xpa[:, 0, 29 : H + 1, 1 : W + 1], xin[:, 28:H, :], SIGN
                        )
                    else:
                        nc.scalar.activation(
                            xpa[:, 0, 1 : H + 1, 1 : W + 1], xin[:], SIGN
                        )
                    # bf16-bitcast flat views so copies run in fast DVE modes
                    fa = xpa[:].rearrange("p a b c -> p (a b c)").bitcast(
                        mybir.dt.bfloat16
                    )  # [128, 3600]
                    fb = xpb[:].rearrange("p a b c -> p (a b c)").bitcast(
                        mybir.dt.bfloat16
                    )
                    # A slot1 = A slot0 shifted +60 fp8 (one row)
                    nc.vector.tensor_copy(fa[:, 1800:3540], fa[:, 30:1770])
                    # B slot0 = A slot0;  B slot1 = A slot0 shifted +2 fp8
                    nc.vector.tensor_copy(fb[:, 0:1740], fa[:, 0:1740])
                    nc.vector.tensor_copy(fb[:, 1800:3540], fa[:, 1:1741])

                    zslot, j = zp[cg // 2], cg % 2
                    # chunk pairs share one 2-bank PSUM tile so the BN1+Sign
                    # eviction reads 2 banks in a single ScalarE op.  Pass
                    # loop is OUTER so each stationary serves both members
                    # back-to-back (halves LDWEIGHTS traffic).
                    for pg in range(4):
                        members = [2 * pg, 2 * pg + 1] if pg < 3 else [6]
                        ps2 = psdw_pool.tile([128, 2, 512], F32, tag="psdw")
                        # (weight idx, buffer, row off, col off) per pass
                        passes = [
                            (cg * 5 + 0, xpa, 0, 0),
                            (cg * 5 + 1, xpa, 0, 1),
                            (cg * 5 + 2, xpa, 0, 2),
                            (cg * 5 + 3, xpb, 2, 0),
                            (cg * 5 + 4, xpa, 2, 1),
                        ]
                        for p, (wi, buf, ro, co) in enumerate(passes):
                            for s, n in enumerate(members):
                                r0 = n * ROWS + ro
                                nc.tensor.matmul(
                                    ps2[:, s, 0 : ROWS * W],
                                    wdw_t[:, wi],
                                    buf[:, :, r0 : r0 + ROWS, co : co + W],
                                    start=(p == 0),
                                    stop=(p == 4),
                                    perf_mode=DR,
                                )
                        r0 = members[0] * ROWS
                        nrows = ROWS * len(members)
                        zout = zslot[:, j, r0 : r0 + nrows, :].rearrange(
                            "p (a r) w -> p a (r w)", a=len(members)
                        )
                        nc.scalar.activation(
                            zout,
                            ps2[:, 0 : len(members), 0 : ROWS * W],
                            SIGN,
                            bias=bn1_t[:, cg * 2 + 1 : cg * 2 + 2],
                            scale=bn1_t[:, cg * 2 : cg * 2 + 1],
                        )

                for cob in range(CG):
                    outb = out_pool.tile([128, H, W], F32, tag="outb")
                    for n in range(NCHUNK):
                        pp = pspw_pool.tile([128, 512], F32, tag="pspw")
                        r0 = n * ROWS
                        for zpair in range(2):
                            nc.tensor.matmul(
                                pp[:, 0 : ROWS * W],
                                wpw_t[:, zpair * CG + cob],
                                zp[zpair][:, :, r0 : r0 + ROWS, :],
                                start=(zpair == 0),
                                stop=(zpair == 1),
                                perf_mode=DR,
                            )
                        oout = outb[:, r0 : r0 + ROWS, :].rearrange(
                            "p r w -> p (r w)"
                        )
                        nc.vector.tensor_scalar(
                            oout,
                            pp[:, 0 : ROWS * W],
                            bn2_t[:, cob * 2 : cob * 2 + 1],
                            bn2_t[:, cob * 2 + 1 : cob * 2 + 2],
                            MULT,
                            ADD,
                        )
                        # stream the output out in halves (rows 0:32 after the
                        # fourth chunk, rest at the end) so the final drain
                        # overlaps compute
                        if n == 3:
                            nc_half = y_d.ap()[b, cob * 128 : (cob + 1) * 128]
                            nc.sync.dma_start(nc_half[:, 0:32, :], outb[:, 0:32, :])
                    nc.sync.dma_start(
                        y_d.ap()[b, cob * 128 : (cob + 1) * 128][:, 32:H, :],
                        outb[:, 32:H, :],
                    )

    _legalize_sem_waits(nc)
    return nc


_NC_CACHE = None


def _get_nc():
    global _NC_CACHE
    if _NC_CACHE is None:
        _NC_CACHE = build_bass()
    return _NC_CACHE


def make_host_inputs(w_dw, w_pw, g1, b1, m1, v1, g2, b2, m2, v2):
    """Host-side preprocessing shared by all cores (weights/BN constants)."""
    wsign = np.sign(w_dw[:, 0, :, :]).reshape(C, 3, 3).astype(np.float32)

    wdw = np.zeros((128, CG * 5, 2, 128), dtype=NP_FP8)
    idx = np.arange(128)
    for cg in range(CG):
        cs = slice(cg * 128, (cg + 1) * 128)
        for dw in range(3):
            wdw[idx, cg * 5 + dw, 0, idx] = wsign[cs, 0, dw].astype(NP_FP8)
            wdw[idx, cg * 5 + dw, 1, idx] = wsign[cs, 1, dw].astype(NP_FP8)
        # pair 3 (buffer B): slot0 = tap (2,0), slot1 = tap (2,2)
        wdw[idx, cg * 5 + 3, 0, idx] = wsign[cs, 2, 0].astype(NP_FP8)
        wdw[idx, cg * 5 + 3, 1, idx] = wsign[cs, 2, 2].astype(NP_FP8)
        # pair 4 (buffer A): slot0 = tap (2,1), slot1 stays zero
        wdw[idx, cg * 5 + 4, 0, idx] = wsign[cs, 2, 1].astype(NP_FP8)

    wptT = np.sign(w_pw[:, :, 0, 0]).T.astype(np.float32)  # [c, co]
    wpw = np.zeros((128, 2 * CG, 2, 128), dtype=NP_FP8)
    for zpair in range(2):
        for cob in range(CG):
            for j in range(2):
                c0 = (zpair * 2 + j) * 128
                wpw[:, zpair * CG + cob, j, :] = wptT[
                    c0 : c0 + 128, cob * 128 : (cob + 1) * 128
                ].astype(NP_FP8)

    def bn_consts(g, bta, m, v):
        s = (g.astype(np.float64) / np.sqrt(v.astype(np.float64) + EPS)).astype(
            np.float32
        )
        t = bta.astype(np.float32) - m.astype(np.float32) * s
        out = np.zeros((128, 2 * CG), dtype=np.float32)
        for cg in range(CG):
            out[:, cg * 2] = s[cg * 128 : (cg + 1) * 128]
            out[:, cg * 2 + 1] = t[cg * 128 : (cg + 1) * 128]
        return out

    return {
        "wdw": wdw,
        "wpw": wpw,
        "bn1": bn_consts(g1, b1, m1, v1),
        "bn2": bn_consts(g2, b2, m2, v2),
    }


def kernel(x, w_dw, w_pw, g1, b1, m1, v1, g2, b2, m2, v2, _trace=False, _tmpdir=None):
    x = np.asarray(x, dtype=np.float32)
    shared = make_host_inputs(
        np.asarray(w_dw), np.asarray(w_pw),
        np.asarray(g1), np.asarray(b1), np.asarray(m1), np.asarray(v1),
        np.asarray(g2), np.asarray(b2), np.asarray(m2), np.asarray(v2),
    )
    in_maps = []
    for i in range(N_CORES):
        m = {"x": np.ascontiguousarray(x[i * BS : (i + 1) * BS])}
        m.update(shared)
        in_maps.append(m)

    nc = _get_nc()
    res = run_bass_kernel_spmd(
        nc, in_maps, core_ids=list(range(N_CORES)), trace=_trace, tmpdir=_tmpdir
    )
    y = np.concatenate([res.results[i]["y"] for i in range(N_CORES)], axis=0)
    if _trace:
        return y, res
    return y
